# revision 26
# baseline (speedup 1.0000x reference)
"""DIN-attention Trainium2 kernel.

out[b] = softmax_t(MLP(concat[q, k, q-k, q*k]) / sqrt(H), mask=t<len_b) @ keys[b]

Strategy (8-core data parallel over B, one shared SPMD program):
- Host sorts b by keys_length, deals round-robin to cores -> per-core slot s
  holds similar lengths on every core; per 16-slot sub-block, work is
  truncated to the sub-block max length (halves all work in expectation).
- MLP decomposition: din@W1 = q@Wq + k@Wk + (q*k)@Wqk with
  Wq=W1a+W1c, Wk=W1b-W1c, Wqk=W1d; the q-term enters the PSUM group via a
  stride-0 broadcast rhs matmul, so relu bias is just b1.
- Scores (M=1 matmuls) are packed 4-per-PSUM-bank at partitions {0,32,64,96},
  assembled to a [128, Tg] tile by one small SBUF->SBUF DMA per sub-block,
  then a 128-row batched masked softmax.
- Output contraction runs on the PE: attn rows are PE-transposed (bf16) and
  used as N=1 moving operands against host-packed bf16 native-layout keys,
  accumulating per-slot output columns in one PSUM bank.
"""

import os
import sys
from contextlib import ExitStack

for _p in ("/opt/trn_rl_repo",):
    if _p not in sys.path:
        sys.path.insert(0, _p)

os.environ.setdefault("CONCOURSE_ENABLE_LDW_OPT", "false")

import numpy as np
import ml_dtypes

import concourse.bass as bass
import concourse.tile as tile
from concourse import bacc, mybir
from concourse.masks import make_identity

F32 = mybir.dt.float32
BF16 = mybir.dt.bfloat16
A = mybir.AluOpType
AF = mybir.ActivationFunctionType

B, T, H = 2048, 200, 128
H1, H2 = 80, 40
NC = 8
SLOTS = B // NC          # 256 slots per core
SB = 16                  # slots per sub-block
NSB = SLOTS // SB        # 16 sub-blocks per core
GROUP_SBS = 4            # sub-blocks per softmax group (4*16 = 64 rows)
NGROUPS = NSB // GROUP_SBS
SCALE = float(1.0 / np.sqrt(np.float32(H)))
NEG = -1e9


def _roundup(x, m):
    return ((int(x) + m - 1) // m) * m


def make_plan(keys_length):
    """Global plan shared by all cores: slot assignment + per-sub-block T."""
    order = np.argsort(keys_length, kind="stable")
    bmap = order.reshape(SLOTS, NC)          # [slot, core] -> b
    lens_slot = np.asarray(keys_length)[bmap]  # [slot, core]
    t_sbs = []
    for sb in range(NSB):
        m = int(lens_slot[sb * SB:(sb + 1) * SB].max())
        t_sbs.append(min(T, _roundup(m, 8)))
    nchs = [max(1, -(-t // 128)) for t in t_sbs]
    kt_offs, off = [], 0
    for t in t_sbs:
        kt_offs.append(off)
        off += SB * t
    kt_w = off
    kn_offs, off = [], 0
    for nch in nchs:
        kn_offs.append(off)
        off += SB * nch * 128
    kn_w = off
    tgs = [max(t_sbs[g * GROUP_SBS:(g + 1) * GROUP_SBS]) for g in range(NGROUPS)]
    return dict(bmap=bmap, t_sbs=t_sbs, nchs=nchs, kt_offs=kt_offs, kt_w=kt_w,
                kn_offs=kn_offs, kn_w=kn_w, tgs=tgs)


def _row_of_ssb(ssb):
    # scores-row of a slot within its sub-block (self-inverse permutation),
    # induced by the scores redistribute DMA (k-strided-outer, quad-inner)
    return (ssb % 4) * 4 + ssb // 4


SECTION_MARKS = []


def _mark(nc, label):
    SECTION_MARKS.append((len(nc.inst_map), label))


def build_body(ctx, tc, outs, ins, plan):
    nc = tc.nc
    SECTION_MARKS.clear()
    keysT_d, knat_d, queryT_d, lens_d = ins[:4]
    out_d, = outs
    t_sbs, nchs = plan["t_sbs"], plan["nchs"]
    kt_offs, kn_offs, tgs = plan["kt_offs"], plan["kn_offs"], plan["tgs"]
    maxT = max(t_sbs)
    maxTg = max(tgs)

    singles = ctx.enter_context(tc.tile_pool(name="singles", bufs=1))
    kt_pool = ctx.enter_context(tc.tile_pool(name="kt", bufs=2))
    kn_pool = ctx.enter_context(tc.tile_pool(name="kn", bufs=GROUP_SBS + 1))
    qk_pool = ctx.enter_context(tc.tile_pool(name="qk", bufs=2))
    h1_pool = ctx.enter_context(tc.tile_pool(name="h1", bufs=3))
    h2_pool = ctx.enter_context(tc.tile_pool(name="h2", bufs=3))
    scr_pool = ctx.enter_context(tc.tile_pool(name="scr", bufs=2))
    grp_pool = ctx.enter_context(tc.tile_pool(name="grp", bufs=2))
    at_pool = ctx.enter_context(tc.tile_pool(name="at", bufs=2))
    ps1_pool = ctx.enter_context(tc.tile_pool(name="ps1", bufs=2, space="PSUM"))
    ps2_pool = ctx.enter_context(tc.tile_pool(name="ps2", bufs=1, space="PSUM"))
    psper_pool = ctx.enter_context(tc.tile_pool(name="psper", bufs=1, space="PSUM"))
    pst_pool = ctx.enter_context(tc.tile_pool(name="pst", bufs=1, space="PSUM"))

    # ---- constants ----
    wk = singles.tile([H, H1], BF16, name="wk")
    wqk = singles.tile([H, H1], BF16, name="wqk")
    wq = singles.tile([H, H1], BF16, name="wq")
    w2 = singles.tile([H1, H2], BF16, name="w2")
    wf = singles.tile([H2, 1], BF16, name="wf")
    b1c = singles.tile([H1, 1], F32, name="b1c")
    b2c = singles.tile([H2, 1], F32, name="b2c")
    # weights arrive host-packed in one [H, H1*3 + ...] tensor? keep separate drams
    wk_d, wqk_d, wq_d, w2_d, wf_d, b1_d, b2_d = ins[4:11]
    nc.sync.dma_start(wk[:], wk_d)
    nc.sync.dma_start(wqk[:], wqk_d)
    nc.sync.dma_start(wq[:], wq_d)
    nc.sync.dma_start(w2[:], w2_d)
    nc.sync.dma_start(wf[:], wf_d)
    nc.sync.dma_start(b1c[:], b1_d[:, None])
    nc.sync.dma_start(b2c[:], b2_d[:, None])
    qt = singles.tile([H, SLOTS], BF16, name="qt")
    nc.sync.dma_start(qt[:], queryT_d)
    qtf = singles.tile([H, SLOTS], F32, name="qtf")
    nc.sync.dma_start(qtf[:], ins[11])
    lens = singles.tile([GROUP_SBS * SB, NGROUPS], F32, name="lens")
    nc.sync.dma_start(lens[:], lens_d)
    iota = singles.tile([128, T], F32, name="iota")
    nc.gpsimd.iota(iota[:], pattern=[[1, T]], base=0, channel_multiplier=0,
                   allow_small_or_imprecise_dtypes=True)
    identb = singles.tile([128, 128], BF16, name="identb")
    make_identity(nc, identb[:])
    zeros1 = singles.tile([1, 128], BF16, name="zeros1")
    nc.vector.memset(zeros1[:], 0.0)
    dummy512 = singles.tile([1, 512], BF16, name="dummy512")
    nc.vector.memset(dummy512[:], 0.0)
    # persistent, one-time-zeroed PSUM banks (manual double buffering)
    pss_t = [psper_pool.tile([128, 512], F32, tag=f"pssp{i}", name=f"pssp{i}")
             for i in range(2)]
    pso_t = [psper_pool.tile([128, 512], F32, tag=f"psop{i}", name=f"psop{i}")
             for i in range(2)]
    for t_ in pss_t + pso_t:
        nc.tensor.matmul(t_[:], zeros1[:], dummy512[:], start=True, stop=True)
    pss_ctr = [0]

    qt_pitch = qt[:].ap[0][0]

    for g in range(NGROUPS):
        tg = tgs[g]
        gslots = GROUP_SBS * SB  # 128
        scores = grp_pool.tile([gslots, tg], F32, tag="scores", name=f"scores_g{g}")
        nc.vector.memset(scores[:], NEG)
        knats = {}
        for isb in range(GROUP_SBS):
            sb = g * GROUP_SBS + isb
            tsb, nch = t_sbs[sb], nchs[sb]
            ns = min(SB, max(1, 512 // tsb))
            _mark(nc, 'dma_kt')
            kt = kt_pool.tile([H, SB * tsb], BF16, tag="kt", name=f"kt_{sb}")
            nc.sync.dma_start(kt[:], keysT_d[:, kt_offs[sb]:kt_offs[sb] + SB * tsb])
            _mark(nc, 'dma_kn')
            kn = kn_pool.tile([128, SB * nch * 128], BF16, tag="kn", name=f"kn_{sb}")
            nc.sync.dma_start(kn[:], knat_d[:, kn_offs[sb]:kn_offs[sb] + SB * nch * 128])
            knats[sb] = (kn, nch)
            _mark(nc, 'qk')
            qt_rep = qk_pool.tile([H, SB * tsb], BF16, tag="qtr", name=f"qtr_{sb}")
            qr_in = bass.AP(tensor=qt[:].tensor, offset=sb * SB,
                            ap=[[qt_pitch, H], [1, SB], [0, tsb]])
            qr_out = bass.AP(tensor=qt_rep[:].tensor, offset=qt_rep[:].offset,
                             ap=[[qt_rep[:].ap[0][0], H], [tsb, SB], [1, tsb]])
            nc.vector.tensor_copy(qr_out, qr_in)
            qkt = qk_pool.tile([H, SB * tsb], BF16, tag="qk", name=f"qk_{sb}")
            nc.vector.tensor_tensor(qkt[:], kt[:], qt_rep[:], op=A.mult)
            scratch = scr_pool.tile([128, 4 * tsb], F32, tag="scr", name=f"scr_{sb}")
            ci = 0
            ps_s = None
            slot0 = 0
            while slot0 < SB:
                ns_c = min(ns, SB - slot0)
                cols = ns_c * tsb
                coff = slot0 * tsb
                _mark(nc, 'm1')
                ps1 = ps1_pool.tile([H1, cols], F32, tag="ps1", name=f"ps1_{sb}_{ci}")
                nc.tensor.matmul(ps1[:], wk[:], kt[:, coff:coff + cols],
                                 start=True, stop=False)
                nc.tensor.matmul(ps1[:], wqk[:], qkt[:, coff:coff + cols],
                                 start=False, stop=False)
                nc.tensor.matmul(ps1[:], wq[:], qt_rep[:, coff:coff + cols],
                                 start=False, stop=True)
                _mark(nc, 'relu1')
                h1 = h1_pool.tile([H1, cols], BF16, tag="h1", name=f"h1_{sb}_{ci}")
                nc.scalar.activation(h1[:], ps1[:], AF.Relu, bias=b1c[:, 0:1],
                                     scale=1.0)
                _mark(nc, 'm2')
                ps2 = ps2_pool.tile([H2, cols], F32, tag="ps2", name=f"ps2_{sb}_{ci}")
                nc.tensor.matmul(ps2[:], w2[:], h1[:], start=True, stop=True)
                _mark(nc, 'relu2')
                h2 = h2_pool.tile([H2, cols], BF16, tag="h2", name=f"h2_{sb}_{ci}")
                nc.vector.tensor_scalar(h2[:], ps2[:], b2c[:, 0:1], 0.0,
                                        op0=A.add, op1=A.max)
                _mark(nc, 'm3')
                npq = 4 if tsb <= 128 else 2  # quads packed per scores bank
                for si in range(ns_c):
                    ssb = slot0 + si
                    q4, k4 = ssb // 4, ssb % 4
                    if ssb % (4 * npq) == 0:
                        ps_s = pss_t[pss_ctr[0] % 2][:, 0:npq * tsb]
                        pss_ctr[0] += 1
                    qq = q4 % npq
                    nc.tensor.matmul(ps_s[32 * k4:32 * k4 + 1,
                                          qq * tsb:(qq + 1) * tsb], wf[:],
                                     h2[:, si * tsb:(si + 1) * tsb],
                                     start=True, stop=True,
                                     tile_position=(0, 32 * k4),
                                     skip_group_check=True)
                    if ssb % (4 * npq) == 4 * npq - 1:
                        dst = scratch[:, (q4 - npq + 1) * tsb:(q4 + 1) * tsb]
                        if q4 % 2 == 0:
                            nc.scalar.copy(dst, ps_s)
                        else:
                            nc.vector.tensor_copy(dst, ps_s)
                slot0 += ns_c
                ci += 1
            _mark(nc, 'redis_s')
            # redistribute scratch -> scores rows [16*isb : 16*isb+16]
            scr_pitch = scratch[:].ap[0][0]
            src = bass.AP(tensor=scratch[:].tensor, offset=scratch[:].offset,
                          ap=[[32 * scr_pitch, 4], [tsb, 4], [1, tsb]])
            nc.sync.dma_start(scores[16 * isb:16 * isb + 16, 0:tsb], src)

        _mark(nc, 'softmax')
        # ---- batched softmax over the group ----
        rmax = grp_pool.tile([gslots, 1], F32, tag="rmax", name=f"rmax_{g}")
        nc.vector.reduce_max(rmax[:], scores[:], axis=mybir.AxisListType.X)
        mexp = grp_pool.tile([gslots, 1], F32, tag="mexp", name=f"mexp_{g}")
        nc.vector.tensor_scalar_mul(mexp[:], rmax[:], -SCALE)
        pexp = grp_pool.tile([gslots, tg], F32, tag="pexp", name=f"pexp_{g}")
        nc.scalar.activation(pexp[:], scores[:], AF.Exp, bias=mexp[:, 0:1],
                             scale=SCALE)
        mask = grp_pool.tile([gslots, tg], F32, tag="mask", name=f"mask_{g}")
        nc.vector.tensor_scalar(mask[:], iota[0:gslots, 0:tg], lens[:, g:g + 1],
                                None, op0=A.is_lt)
        pm = grp_pool.tile([gslots, tg], F32, tag="pm", name=f"pm_{g}")
        nc.vector.tensor_tensor(pm[:], pexp[:], mask[:], op=A.mult)
        zsum = grp_pool.tile([gslots, 1], F32, tag="zsum", name=f"zsum_{g}")
        nc.vector.reduce_sum(zsum[:], pm[:], axis=mybir.AxisListType.X)
        rz = grp_pool.tile([gslots, 1], F32, tag="rz", name=f"rz_{g}")
        nc.vector.reciprocal(rz[:], zsum[:])
        attnb = grp_pool.tile([gslots, tg], BF16, tag="attnb", name=f"attnb_{g}")
        nc.vector.tensor_scalar_mul(attnb[:], pm[:], rz[:, 0:1])

        _mark(nc, 'transpose')
        # ---- transpose attn (bf16) in 128-col chunks ----
        atts = []
        for c in range(-(-tg // 128)):
            cl = min(128, tg - 128 * c)
            ps_t = pst_pool.tile([cl, gslots], BF16, tag="pst", name=f"pst_{g}_{c}")
            nc.tensor.transpose(ps_t[:], attnb[:, 128 * c:128 * c + cl],
                                identb[0:gslots, 0:gslots])
            at = at_pool.tile([cl, gslots], BF16, tag="at", name=f"at_{g}_{c}")
            nc.vector.tensor_copy(at[:], ps_t[:])
            atts.append(at)

        _mark(nc, 'final')
        # ---- final contraction ----
        # Per slot: lhsT = its attn column (M=1, ~free LDW), rhs = bf16 native
        # key chunk (N=128). Out rows pack {0,32,64,96} x 4 col-blocks per
        # PSUM bank (16 slots/tile), extracted like the scores.
        for isb in range(GROUP_SBS):
            sb = g * GROUP_SBS + isb
            tsb = t_sbs[sb]
            kn, nch = knats[sb]
            oscr = scr_pool.tile([128, 512], F32, tag="oscr", name=f"oscr_{sb}")
            ps_o = pso_t[sb % 2]
            for ssb in range(SB):
                r = 16 * isb + _row_of_ssb(ssb)
                k4, q4 = ssb % 4, ssb // 4
                for c in range(nch):
                    cl = min(128, tsb - 128 * c)
                    blk = (ssb * nch + c) * 128
                    nc.tensor.matmul(
                        ps_o[32 * k4:32 * k4 + 1, 128 * q4:128 * q4 + 128],
                        atts[c][0:cl, r:r + 1],
                        kn[0:cl, blk:blk + 128],
                        start=(c == 0), stop=(c == nch - 1),
                        tile_position=(0, 32 * k4),
                        skip_group_check=True)
            nc.scalar.copy(oscr[:], ps_o[:])
            # redistribute: oscr[32k, 128q + h] -> out_nat row 16isb + k*4 + q
            os_pitch = oscr[:].ap[0][0]
            src = bass.AP(tensor=oscr[:].tensor, offset=oscr[:].offset,
                          ap=[[32 * os_pitch, 4], [128, 4], [1, 128]])
            nc.sync.dma_start(out_d[g * gslots + 16 * isb:
                                    g * gslots + 16 * isb + 16, :], src)


def pack_inputs(query, keys, keys_length, W1, b1, W2, b2, Wf, bf, plan):
    """Build the 8 per-core input maps. Returns (in_maps, names/shapes list)."""
    bmap, t_sbs, nchs = plan["bmap"], plan["t_sbs"], plan["nchs"]
    kt_w, kn_w = plan["kt_w"], plan["kn_w"]
    Wq = (W1[0:H] + W1[2 * H:3 * H]).astype(np.float32)
    Wk = (W1[H:2 * H] - W1[2 * H:3 * H]).astype(np.float32)
    Wqk = W1[3 * H:4 * H].astype(np.float32)
    in_maps = []
    for c in range(NC):
        ktp = np.zeros((H, kt_w), ml_dtypes.bfloat16)
        knp = np.zeros((128, kn_w), ml_dtypes.bfloat16)
        qtp = np.zeros((H, SLOTS), ml_dtypes.bfloat16)
        lensp = np.zeros((GROUP_SBS * SB, NGROUPS), np.float32)
        for sb in range(NSB):
            tsb, nch = t_sbs[sb], nchs[sb]
            ko, no = plan["kt_offs"][sb], plan["kn_offs"][sb]
            g, isb = sb // GROUP_SBS, sb % GROUP_SBS
            for ssb in range(SB):
                s = sb * SB + ssb
                b = int(bmap[s, c])
                ktp[:, ko + ssb * tsb: ko + (ssb + 1) * tsb] = keys[b, :tsb, :].T
                for ch in range(nch):
                    cl = min(128, tsb - 128 * ch)
                    blk = no + (ssb * nch + ch) * 128
                    knp[0:cl, blk:blk + 128] = keys[b, 128 * ch:128 * ch + cl, :]
                qtp[:, s] = query[b]
                lensp[16 * isb + _row_of_ssb(ssb), g] = keys_length[b]
        bf = ml_dtypes.bfloat16
        in_maps.append({"keysT": ktp, "knat": knp, "queryT": qtp, "lens": lensp,
                        "wk": Wk.astype(bf), "wqk": Wqk.astype(bf),
                        "wq": Wq.astype(bf), "w2": W2.astype(bf),
                        "wf": Wf.astype(bf), "b1": b1.astype(np.float32),
                        "b2": b2.astype(np.float32),
                        "queryTf": qtp.astype(np.float32)})
    return in_maps


def build_program(plan):
    nc = bacc.Bacc("TRN2", num_devices=NC)
    ins = [
        nc.dram_tensor("keysT", [H, plan["kt_w"]], BF16, kind="ExternalInput").ap(),
        nc.dram_tensor("knat", [128, plan["kn_w"]], BF16, kind="ExternalInput").ap(),
        nc.dram_tensor("queryT", [H, SLOTS], BF16, kind="ExternalInput").ap(),
        nc.dram_tensor("lens", [GROUP_SBS * SB, NGROUPS], F32,
                       kind="ExternalInput").ap(),
        nc.dram_tensor("wk", [H, H1], BF16, kind="ExternalInput").ap(),
        nc.dram_tensor("wqk", [H, H1], BF16, kind="ExternalInput").ap(),
        nc.dram_tensor("wq", [H, H1], BF16, kind="ExternalInput").ap(),
        nc.dram_tensor("w2", [H1, H2], BF16, kind="ExternalInput").ap(),
        nc.dram_tensor("wf", [H2, 1], BF16, kind="ExternalInput").ap(),
        nc.dram_tensor("b1", [H1], F32, kind="ExternalInput").ap(),
        nc.dram_tensor("b2", [H2], F32, kind="ExternalInput").ap(),
        nc.dram_tensor("queryTf", [H, SLOTS], F32, kind="ExternalInput").ap(),
    ]
    outs = [nc.dram_tensor("outN", [SLOTS, H], F32, kind="ExternalOutput").ap()]
    with tile.TileContext(nc) as tc:
        with ExitStack() as ctx:
            build_body(ctx, tc, outs, ins, plan)
    nc.compile()
    return nc


last_results = None  # stash for external profiling/analysis


def kernel(query, keys, keys_length, W1, b1, W2, b2, Wf, bf):
    global last_results
    from concourse.bass_utils import run_bass_kernel_spmd
    query = np.asarray(query, np.float32)
    keys = np.asarray(keys, np.float32)
    keys_length = np.asarray(keys_length)
    plan = make_plan(keys_length)
    in_maps = pack_inputs(query, keys, keys_length, np.asarray(W1, np.float32),
                          np.asarray(b1, np.float32), np.asarray(W2, np.float32),
                          np.asarray(b2, np.float32), np.asarray(Wf, np.float32),
                          np.asarray(bf, np.float32), plan)
    nc = build_program(plan)
    trace = bool(int(os.environ.get("BASS_KERNEL_TRACE", "0")))
    res = run_bass_kernel_spmd(nc, in_maps, core_ids=list(range(NC)), trace=trace)
    last_results = res
    globals()["last_nc"] = nc
    if trace and res.exec_time_ns is not None:
        print(f"HW exec time: {res.exec_time_ns} ns")
    out = np.zeros((B, H), np.float32)
    bmap = plan["bmap"]
    rows = np.array([16 * (s // 16) + _row_of_ssb(s % 16) for s in range(SLOTS)])
    for c in range(NC):
        outN = res.results[c]["outN"]  # [SLOTS, H], row 16*sb + perm(ssb)
        out[bmap[:, c]] = outN[rows]
    return out
